# revision 1
# baseline (speedup 1.0000x reference)
"""MultiHeadAttention Trainium2 kernel.

Sharding: 8 cores = 4 batches x 2 head-groups (8 heads each).
Each core computes, for its (batch b, head group gi):
  Q = q[b] @ Wq[:, gi*512:+512] + bq_g        (and same fc applied to k, v)
  per head: softmax(QK^T/8 with mask) @ V
  partial_out = attn @ Wo[gi*512:+512, :]
Host sums the two partial outputs per batch and adds b_o.

Device layout notes (per core):
  - inputs arrive TRANSPOSED: xT [1024, seq] so projections need no transposes
  - Q^T, K^T stored [128, 4, seq] bf16 (partition = d within d-tile; head g
    occupies partitions 64*(g%2).. of d-tile g//2 -> natural head pairing)
  - scores computed transposed S^T[sk, sq] with two heads packed in the PE
    array via tile_position row tiling (K=64 each)
  - softmax without max-subtraction (scores bounded ~|6| after 1/8 scaling)
  - mask applied multiplicatively AFTER exp (notmask in {0,1} bf16)
  - attn@V uses lhsT=[V_head | ones] (M=65): row 64 accumulates the softmax
    denominator for free
  - normalize via exp(-ln(sum)) on ACT + gpsimd partition_broadcast (custom
    DVE reciprocal ops are broken on HW via this runtime path)
"""

import sys

import numpy as np
import ml_dtypes

try:
    import concourse.bass as bass  # noqa: F401
except ImportError:  # pragma: no cover
    for _p in ("/opt/trn_rl_repo", "/root/.axon_site/_ro/trn_rl_repo"):
        if _p not in sys.path:
            sys.path.insert(0, _p)
    import concourse.bass as bass  # noqa: F401

import concourse.tile as tile
from concourse import bacc, mybir
from concourse.bass_utils import run_bass_kernel_spmd

BF16 = ml_dtypes.bfloat16

D_MODEL = 1024
N_HEADS = 16
BATCH = 4
SEQ = 2048
DH = 64           # head dim
HG = 8            # heads per core
DG = HG * DH      # 512, projected dim per core

F32 = mybir.dt.float32
F32R = mybir.dt.float32r
BF16D = mybir.dt.bfloat16


def build_nc(seq=SEQ):
    """Build the per-core SPMD Bass program."""
    assert seq % 512 == 0
    NT = seq // 128       # sk tiles
    NC_ = seq // 512      # sq chunks
    NST = seq // 128      # s tiles for V / out rows

    nc = bacc.Bacc(None, target_bir_lowering=False)

    xqT = nc.dram_tensor("xqT", [D_MODEL, seq], F32R, kind="ExternalInput")
    xkT = nc.dram_tensor("xkT", [D_MODEL, seq], F32R, kind="ExternalInput")
    xvT = nc.dram_tensor("xvT", [D_MODEL, seq], F32R, kind="ExternalInput")
    wq = nc.dram_tensor("wq", [D_MODEL, DG], F32R, kind="ExternalInput")
    bq = nc.dram_tensor("bq", [DG], F32, kind="ExternalInput")
    wo = nc.dram_tensor("wo", [DG, D_MODEL], BF16D, kind="ExternalInput")
    # notmask, transposed + tiled: [pair, c, t, p(sk in tile), h(in pair), j(sq in chunk)]
    nm = nc.dram_tensor("nm", [4, NC_, NT, 128, 2, 512], BF16D, kind="ExternalInput")
    out = nc.dram_tensor("out", [seq, D_MODEL], F32, kind="ExternalOutput")

    EXP = mybir.ActivationFunctionType.Exp
    LN = mybir.ActivationFunctionType.Ln
    IDENT = mybir.ActivationFunctionType.Identity

    with tile.TileContext(nc) as tc:
        with tc.tile_pool(name="persist", bufs=1) as persist:
            qt_sb = persist.tile([128, 4, seq], BF16D, name="qt_sb")
            kt_sb = persist.tile([128, 4, seq], BF16D, name="kt_sb")
            v_sb = persist.tile([128, NST, HG, DH + 1], BF16D, name="v_sb")
            wo_sb = persist.tile([128, 4, D_MODEL], BF16D, name="wo_sb")
            attnT = persist.tile([128, 4, seq], BF16D, name="attnT")
            bq_sb = persist.tile([128, 4], F32, name="bq_sb")
            bqrep = persist.tile([128, HG, DH], F32, name="bqrep")

            # ---------------- Phase A: projections ----------------
            with tc.tile_pool(name="xpool", bufs=12) as xpool, \
                 tc.tile_pool(name="wqpool", bufs=1) as wqp, \
                 tc.tile_pool(name="projps", bufs=2, space="PSUM") as projps:

                wq_sb = wqp.tile([128, 8, DG], F32R, name="wq_sb")
                nc.sync.dma_start(out=wq_sb, in_=wq.rearrange("(n p) m -> p n m", p=128))
                nc.sync.dma_start(out=bq_sb, in_=bq.rearrange("(t p) -> p t", p=128))
                _bqap = bq[:].rearrange("(g e) -> g e", g=HG)
                nc.gpsimd.dma_start(out=bqrep, in_=bass.AP(
                    tensor=_bqap.tensor, offset=_bqap.offset,
                    ap=[[0, 128]] + [list(d) for d in _bqap.ap]))
                nc.sync.dma_start(out=wo_sb, in_=wo.rearrange("(n p) m -> p n m", p=128))
                nc.vector.memset(v_sb[:, :, :, DH:DH + 1], 1.0)

                def load_x(xh, pref):
                    ts_ = []
                    for db in range(8):
                        xt = xpool.tile([128, seq], F32R, name=f"{pref}{db}", tag="x")
                        nc.sync.dma_start(out=xt, in_=xh[db * 128:(db + 1) * 128, :])
                        ts_.append(xt)
                    return ts_

                # Q^T and K^T: out[d_tile, sq] accumulated over D blocks
                for xh, dst in ((xqT, qt_sb), (xkT, kt_sb)):
                    xts = load_x(xh, "xq" if dst is qt_sb else "xk")
                    for dt in range(4):
                        for h0 in range(0, seq, 1024):
                            cw = min(1024, seq - h0)
                            ps = projps.tile([128, cw], F32, name="pps", tag="pps")
                            for db in range(8):
                                for j0 in range(0, cw, 512):
                                    nc.tensor.matmul(
                                        ps[:, j0:j0 + 512],
                                        wq_sb[:, db, dt * 128:(dt + 1) * 128],
                                        xts[db][:, h0 + j0: h0 + j0 + 512],
                                        start=(db == 0), stop=(db == 7),
                                    )
                            nc.scalar.activation(
                                dst[:, dt, h0:h0 + cw], ps,
                                IDENT, bias=bq_sb[:, dt:dt + 1], scale=1.0,
                            )
                # V: out[s_tile, d] accumulated over D blocks
                xts = load_x(xvT, "xv")
                for st in range(NST):
                    ps = projps.tile([128, DG], F32, name="pps", tag="pps")
                    for db in range(8):
                        nc.tensor.matmul(
                            ps, xts[db][:, st * 128:(st + 1) * 128], wq_sb[:, db, :],
                            start=(db == 0), stop=(db == 7),
                        )
                    nc.vector.tensor_add(
                        v_sb[:, st, :, 0:DH],
                        ps[:, :].rearrange("p (g e) -> p g e", g=HG),
                        bqrep,
                    )

            # ---------------- Phase B: attention ----------------
            with tc.tile_pool(name="nmp", bufs=10) as nmp, \
                 tc.tile_pool(name="probsp", bufs=4) as probsp, \
                 tc.tile_pool(name="rsbp", bufs=2) as rsbp, \
                 tc.tile_pool(name="rrepp", bufs=2) as rrepp, \
                 tc.tile_pool(name="tmpp", bufs=2) as tmpp, \
                 tc.tile_pool(name="osbp", bufs=3) as osbp, \
                 tc.tile_pool(name="spairp", bufs=2, space="PSUM") as spairp, \
                 tc.tile_pool(name="accp", bufs=1, space="PSUM") as accp, \
                 tc.tile_pool(name="outpsp", bufs=2, space="PSUM") as outpsp:

                for c in range(NC_):
                    cs = slice(c * 512, (c + 1) * 512)
                    for pr in range(4):
                        acc = accp.tile([DH + 1, 2, 512], F32, name="acc", tag="acc")
                        for t in range(NT):
                            nmt = nmp.tile([128, 2, 512], BF16D, name="nmt", tag="nmt")
                            nc.sync.dma_start(out=nmt, in_=nm[pr, c, t])
                            spair = spairp.tile([128, 2, 512], F32, name="spair", tag="spair")
                            tc_cols = slice(t * 128, (t + 1) * 128)
                            nc.tensor.matmul(
                                spair[:, 0, :], kt_sb[0:64, pr, tc_cols],
                                qt_sb[0:64, pr, cs], start=True, stop=True,
                                tile_position=(0, 0),
                            )
                            nc.tensor.matmul(
                                spair[:, 1, :], kt_sb[64:128, pr, tc_cols],
                                qt_sb[64:128, pr, cs], start=True, stop=True,
                                tile_position=(64, 0),
                            )
                            probs = probsp.tile([128, 2, 512], BF16D, name="probs", tag="probs")
                            nc.scalar.activation(probs, spair, EXP, scale=0.125)
                            nc.vector.tensor_mul(probs, probs, nmt)
                            for h in range(2):
                                nc.tensor.matmul(
                                    acc[:, h, :], v_sb[:, t, 2 * pr + h, :],
                                    probs[:, h, :], start=(t == 0), stop=(t == NT - 1),
                                )
                        # normalize this pair's chunk
                        lnt = rsbp.tile([1, 2, 512], F32, name="lnt", tag="lnt")
                        nc.scalar.activation(lnt, acc[DH:DH + 1, :, :], LN, scale=1.0)
                        rsb = rsbp.tile([1, 2, 512], F32, name="rsb", tag="rsb")
                        nc.scalar.activation(rsb, lnt, EXP, scale=-1.0)
                        rrep = rrepp.tile([64, 2, 512], F32, name="rrep", tag="rrep")
                        nc.gpsimd.partition_broadcast(rrep, rsb)
                        tmpt = tmpp.tile([64, 2, 512], BF16D, name="tmpt", tag="tmpt")
                        nc.vector.tensor_mul(tmpt, acc[0:DH, :, :], rrep)
                        nc.sync.dma_start(out=attnT[0:64, pr, cs], in_=tmpt[:, 0, :])
                        nc.sync.dma_start(out=attnT[64:128, pr, cs], in_=tmpt[:, 1, :])
                    # output projection for this sq chunk
                    for k in range(4):
                        row = c * 512 + k * 128
                        for dch in range(2):
                            ops = outpsp.tile([128, 512], F32, name="ops", tag="ops")
                            for pr in range(4):
                                nc.tensor.matmul(
                                    ops, attnT[:, pr, row:row + 128],
                                    wo_sb[:, pr, dch * 512:(dch + 1) * 512],
                                    start=(pr == 0), stop=(pr == 3),
                                )
                            osb = osbp.tile([128, 512], F32, name="osb", tag="osb")
                            nc.vector.tensor_copy(osb, ops)
                            nc.sync.dma_start(
                                out=out[row:row + 128, dch * 512:(dch + 1) * 512], in_=osb
                            )

    nc.compile()
    return nc


_NC_CACHE = {}


def _get_nc(seq=SEQ):
    if seq not in _NC_CACHE:
        _NC_CACHE[seq] = build_nc(seq)
    return _NC_CACHE[seq]


def make_core_inputs(q, k, v, mask, W_q, b_q, W_o, seq=SEQ):
    """Build the 8 per-core input maps (host-side shard + layout)."""
    NT = seq // 128
    NC_ = seq // 512
    in_maps = []
    notm_all = (~np.asarray(mask)).astype(BF16)  # [B, 16, sq, sk]
    for core in range(8):
        b, gi = divmod(core, 2)
        cols = slice(gi * DG, (gi + 1) * DG)
        xqT = np.ascontiguousarray(np.asarray(q[b], np.float32).T)
        xkT = np.ascontiguousarray(np.asarray(k[b], np.float32).T)
        xvT = np.ascontiguousarray(np.asarray(v[b], np.float32).T)
        wqc = np.ascontiguousarray(np.asarray(W_q, np.float32)[:, cols])
        bqc = np.ascontiguousarray(np.asarray(b_q, np.float32)[cols])
        woc = np.ascontiguousarray(np.asarray(W_o, np.float32)[cols, :]).astype(BF16)
        nmc = notm_all[b, gi * HG:(gi + 1) * HG]  # [8, sq, sk] bf16
        # -> [pair, c, t, p, h, j]
        nmc = np.ascontiguousarray(
            nmc.reshape(4, 2, NC_, 512, NT, 128).transpose(0, 2, 4, 5, 1, 3)
        )
        in_maps.append({
            "xqT": xqT, "xkT": xkT, "xvT": xvT,
            "wq": wqc, "bq": bqc, "wo": woc, "nm": nmc,
        })
    return in_maps


def kernel(q, k, v, mask, W_q, b_q, W_o, b_o):
    nc = _get_nc(SEQ)
    in_maps = make_core_inputs(q, k, v, mask, W_q, b_q, W_o, SEQ)
    res = run_bass_kernel_spmd(nc, in_maps, core_ids=list(range(8)))
    out = np.empty((BATCH, SEQ, D_MODEL), np.float32)
    bo = np.asarray(b_o, np.float32)
    for b in range(BATCH):
        out[b] = res.results[2 * b]["out"] + res.results[2 * b + 1]["out"] + bo
    return out



# revision 2
# speedup vs baseline: 1.6078x; 1.6078x over previous
"""MultiHeadAttention Trainium2 kernel, v2.

Sharding: 8 cores = 4 batches x 2 head-groups (8 heads each).
Per core (batch b, head group gi):
  Q = q[b] @ Wq[:, gi*512:+512] + bq_g   (same fc applied to k, v)
  per head: softmax(QK^T/8 with mask) @ V
  partial_out = attn @ Wo[gi*512:+512, :]
Host sums the two partial outputs per batch and adds b_o.

v2 design (vs v1 baseline):
  - all inputs bf16 (halves x DMA; DMA_ENGINES is a serialized shared
    resource in the cost model, ~294us in v1)
  - x loaded in column-chunk tiles [128, 8db, 512] (24KB SBUF, one pool)
  - scores S^T[sk, sq] as before, but attn@V runs TRANSPOSED: probs is
    the stationary operand, out acc[sq_part, 65] -> pays 65 output rows
    per matmul instead of 512 (cost model charges out free-size only).
    All 16 probs tiles of a (c,pr) stay in SBUF; the 8 (h,j) psum
    accumulation groups then run sequentially over t (one open group
    per psum bank), software-pipelined one (c,pr) behind scores/exp.
  - 65th column of V = ones accumulates the softmax denominator, which
    lands as a PER-PARTITION scalar -> normalize via
    nc.vector.reciprocal (IEEE 1/x on trn2) + tensor_scalar mult on
    DVE. Kills v1's exp(-ln(sum)) on ACT, its LoadActFuncSet thrash
    (33 table loads = 42us), and the gpsimd broadcast.
  - attnT assembled via DMA-transpose XBAR (14ns/tile) instead of PE
  - ACT engine does ONLY the exp (the hard floor: 256 x ~1038ns)
  - Q/K bias-adds moved to DVE tensor_scalar (PSUM->SBUF copy + bias)
  - out stores bf16 via gpsimd (Pool) SWDGE queue; host sums in f32
  - phase A emits K^T, Q chunk 0, V tiles 0..3 up front; remaining V
    tiles and Q chunks are emitted just-in-time inside phase B
"""

import sys

import numpy as np
import ml_dtypes

try:
    import concourse.bass as bass  # noqa: F401
except ImportError:  # pragma: no cover
    for _p in ("/opt/trn_rl_repo", "/root/.axon_site/_ro/trn_rl_repo"):
        if _p not in sys.path:
            sys.path.insert(0, _p)
    import concourse.bass as bass  # noqa: F401

import concourse.tile as tile
from concourse import bacc, mybir
from concourse.bass_utils import run_bass_kernel_spmd

BF16 = ml_dtypes.bfloat16

D_MODEL = 1024
N_HEADS = 16
BATCH = 4
SEQ = 2048
DH = 64           # head dim
HG = 8            # heads per core
DG = HG * DH      # 512, projected dim per core

F32 = mybir.dt.float32
BF16D = mybir.dt.bfloat16


def build_nc(seq=SEQ):
    """Build the per-core SPMD Bass program."""
    assert seq % 512 == 0
    NT = seq // 128       # sk tiles
    NC_ = seq // 512      # sq chunks
    NST = seq // 128      # s tiles for V

    nc = bacc.Bacc(None, target_bir_lowering=False)

    xqT = nc.dram_tensor("xqT", [D_MODEL, seq], BF16D, kind="ExternalInput")
    xkT = nc.dram_tensor("xkT", [D_MODEL, seq], BF16D, kind="ExternalInput")
    xvT = nc.dram_tensor("xvT", [D_MODEL, seq], BF16D, kind="ExternalInput")
    wq = nc.dram_tensor("wq", [D_MODEL, DG], BF16D, kind="ExternalInput")
    bq = nc.dram_tensor("bq", [DG], F32, kind="ExternalInput")
    wo = nc.dram_tensor("wo", [DG, D_MODEL], BF16D, kind="ExternalInput")
    # notmask, transposed + tiled: [pair, c, t, p(sk in tile), h(in pair), j(sq in chunk)]
    nm = nc.dram_tensor("nm", [4, NC_, NT, 128, 2, 512], BF16D, kind="ExternalInput")
    out = nc.dram_tensor("out", [seq, D_MODEL], BF16D, kind="ExternalOutput")

    EXP = mybir.ActivationFunctionType.Exp
    ADD = mybir.AluOpType.add
    MULT = mybir.AluOpType.mult

    with tile.TileContext(nc) as tc:
        with tc.tile_pool(name="persist", bufs=1) as persist:
            kt_sb = persist.tile([128, 4, seq], BF16D, name="kt_sb")
            v_sb = persist.tile([128, NST, HG, DH + 1], BF16D, name="v_sb")
            wo_sb = persist.tile([128, 4, D_MODEL], BF16D, name="wo_sb")
            wq_sb = persist.tile([128, 8, DG], BF16D, name="wq_sb")
            bq_sb = persist.tile([128, 4], F32, name="bq_sb")
            bqrep = persist.tile([128, HG, DH], F32, name="bqrep")

            with tc.tile_pool(name="xkp", bufs=4) as xkp, \
                 tc.tile_pool(name="xqp", bufs=2) as xqp, \
                 tc.tile_pool(name="xvp", bufs=2) as xvp, \
                 tc.tile_pool(name="qp", bufs=3) as qp, \
                 tc.tile_pool(name="ps512", bufs=2, space="PSUM") as ps512, \
                 tc.tile_pool(name="nmp", bufs=5) as nmp, \
                 tc.tile_pool(name="probsp", bufs=33) as probsp, \
                 tc.tile_pool(name="recp", bufs=3) as recp, \
                 tc.tile_pool(name="asbp", bufs=3) as asbp, \
                 tc.tile_pool(name="attnTp", bufs=2) as attnTp, \
                 tc.tile_pool(name="osbp", bufs=3) as osbp, \
                 tc.tile_pool(name="spairp", bufs=2, space="PSUM") as spairp, \
                 tc.tile_pool(name="accp", bufs=2, space="PSUM") as accp:

                # ---- weights / bias loads (ACT hwdge queue) ----
                nc.scalar.dma_start(out=wq_sb, in_=wq.rearrange("(n p) m -> p n m", p=128))
                nc.scalar.dma_start(out=bq_sb, in_=bq.rearrange("(t p) -> p t", p=128))
                _bqap = bq[:].rearrange("(g e) -> g e", g=HG)
                nc.gpsimd.dma_start(out=bqrep, in_=bass.AP(
                    tensor=_bqap.tensor, offset=_bqap.offset,
                    ap=[[0, 128]] + [list(d) for d in _bqap.ap]))
                nc.scalar.dma_start(out=wo_sb, in_=wo.rearrange("(n p) m -> p n m", p=128))
                nc.vector.memset(v_sb[:, :, :, DH:DH + 1], 1.0)

                def load_x_chunk(pool, xh, c, pref):
                    """[128 p(D in block), 8 db, 512 s] from xT[:, c*512:+512]."""
                    xt = pool.tile([128, 8, 512], BF16D, name=f"{pref}{c}", tag="x")
                    nc.sync.dma_start(
                        out=xt,
                        in_=xh[:, c * 512:(c + 1) * 512].rearrange(
                            "(n p) m -> p n m", p=128),
                    )
                    return xt

                def emit_k(xt, dt, c):
                    """Project K^T dt-slice of one 512-col chunk into kt_sb."""
                    ps = ps512.tile([128, 512], F32, name="ps", tag="ps")
                    for db in range(8):
                        nc.tensor.matmul(
                            ps, wq_sb[:, db, dt * 128:(dt + 1) * 128],
                            xt[:, db, :], start=(db == 0), stop=(db == 7),
                        )
                    nc.vector.tensor_scalar(
                        kt_sb[:, dt, c * 512:(c + 1) * 512], ps,
                        bq_sb[:, dt:dt + 1], None, ADD,
                    )

                def emit_q(xt, dt, c):
                    """Project Q^T dt-slice for chunk c into a fresh q tile."""
                    ps = ps512.tile([128, 512], F32, name="ps", tag="ps")
                    for db in range(8):
                        nc.tensor.matmul(
                            ps, wq_sb[:, db, dt * 128:(dt + 1) * 128],
                            xt[:, db, :], start=(db == 0), stop=(db == 7),
                        )
                    qt = qp.tile([128, 512], BF16D, name="qt", tag="qt")
                    nc.vector.tensor_scalar(qt, ps, bq_sb[:, dt:dt + 1], None, ADD)
                    return qt

                def emit_v(xt, st):
                    """Project V s-tile st: v_sb[:, st, :, 0:64]."""
                    i = st % 4
                    ps = ps512.tile([128, 512], F32, name="ps", tag="ps")
                    for db in range(8):
                        nc.tensor.matmul(
                            ps, xt[:, db, i * 128:(i + 1) * 128], wq_sb[:, db, :],
                            start=(db == 0), stop=(db == 7),
                        )
                    nc.vector.tensor_add(
                        v_sb[:, st, :, 0:DH],
                        ps[:, :].rearrange("p (g e) -> p g e", g=HG),
                        bqrep,
                    )

                def beta_phase(c, pr, probs_list, attnT, j):
                    """attn@V j-phase: 2 psum groups (h0, h1) over all t,
                    then normalize + DMA-transpose into attnT."""
                    asb = asbp.tile([128, 2, DH], BF16D, name="asb", tag="asb")
                    for h in range(2):
                        acc = accp.tile([128, 512], F32, name="acc", tag="acc")
                        for t in range(NT):
                            nc.tensor.matmul(
                                acc[:, 0:DH + 1],
                                probs_list[t][:, h, j * 128:(j + 1) * 128],
                                v_sb[:, t, 2 * pr + h, :],
                                start=(t == 0), stop=(t == NT - 1),
                            )
                        recip = recp.tile([128, 1], F32, name="recip", tag="recip")
                        nc.vector.reciprocal(recip, acc[:, DH:DH + 1])
                        nc.vector.tensor_scalar(
                            asb[:, h, :], acc[:, 0:DH], recip[:, :], None, MULT,
                        )
                    nc.sync.dma_start(
                        out=attnT[:, pr, j * 128:(j + 1) * 128],
                        in_=asb, transpose=True,
                    )

                def outproj_group(c, attnT, k, dch):
                    row = c * 512 + k * 128
                    ops = ps512.tile([128, 512], F32, name="ops", tag="ps")
                    for pr in range(4):
                        nc.tensor.matmul(
                            ops, attnT[:, pr, k * 128:(k + 1) * 128],
                            wo_sb[:, pr, dch * 512:(dch + 1) * 512],
                            start=(pr == 0), stop=(pr == 3),
                        )
                    osb = osbp.tile([128, 512], BF16D, name="osb", tag="osb")
                    nc.vector.tensor_copy(osb, ops)
                    nc.gpsimd.dma_start(
                        out=out[row:row + 128, dch * 512:(dch + 1) * 512],
                        in_=osb,
                    )

                # ---- prefix: minimal work before the first score ----
                xk_tiles = {0: load_x_chunk(xkp, xkT, 0, "xk")}
                xq_tiles = {0: load_x_chunk(xqp, xqT, 0, "xq")}
                xv_tiles = {0: load_x_chunk(xvp, xvT, 0, "xv")}
                emit_k(xk_tiles[0], 0, 0)
                q_next = emit_q(xq_tiles[0], 0, 0)

                # ---- phase B: everything else JIT-interleaved per slot ----
                # pair index i = c*4 + pr, c-major.  Per-slot JIT calendar:
                #   c0: K[dt=pr, ch0] at pair start, ch 1-3 at slots 3,7,11;
                #       pr0 also carries V st0-15 (one per slot).
                #   all pairs: Q for the next pair at slot 13 (xq staged at 9).
                #   outproj(c-1) groups at slots 2,6,10,14 of pr1 and pr2.
                #   beta j-phases of the previous pair at slots 3,7,11,15.
                prev = None  # (c, pr, probs_list, attnT)
                attnT_cur = None
                attnT_hist = {}
                for i in range(NC_ * 4):
                    c, pr = divmod(i, 4)
                    if pr == 0:
                        attnT_cur = attnTp.tile([128, 4, 512], BF16D,
                                                name="attnT", tag="attnT")
                        attnT_hist[c] = attnT_cur
                    if c == 0 and pr >= 1:
                        emit_k(xk_tiles[0], pr, 0)
                    qt = q_next
                    probs_list = []
                    for t in range(NT):
                        # --- JIT calendar for this slot ---
                        if c == 0 and pr == 0:
                            if t in (0, 4, 8):
                                ch = t // 4 + 1
                                xk_tiles[ch] = load_x_chunk(xkp, xkT, ch, "xk")
                                if ch < 3:
                                    xv_tiles[ch] = load_x_chunk(xvp, xvT, ch, "xv")
                            if t == 9:
                                xv_tiles[3] = load_x_chunk(xvp, xvT, 3, "xv")
                            emit_v(xv_tiles[t // 4], t)
                        if c == 0 and t in (3, 7, 11):
                            emit_k(xk_tiles[t // 4 + 1], pr, t // 4 + 1)
                        if t == 9 and i + 1 < NC_ * 4 and (i + 1) % 4 == 0:
                            nc2 = (i + 1) // 4
                            xq_tiles[nc2] = load_x_chunk(xqp, xqT, nc2, "xq")
                        if t == 13 and i + 1 < NC_ * 4:
                            c2, pr2 = divmod(i + 1, 4)
                            q_next = emit_q(xq_tiles[c2], pr2, c2)
                        if c >= 1 and pr in (1, 2) and t in (2, 6, 10, 14):
                            g = (pr - 1) * 4 + t // 4  # 0..7
                            outproj_group(c - 1, attnT_hist[c - 1], g // 2, g % 2)
                        # --- main per-slot pipeline ---
                        nmt = nmp.tile([128, 2, 512], BF16D, name="nmt", tag="nmt")
                        nc.sync.dma_start(out=nmt, in_=nm[pr, c, t])
                        spair = spairp.tile([128, 2, 512], F32, name="spair", tag="spair")
                        tcs = slice(t * 128, (t + 1) * 128)
                        for h in range(2):
                            nc.tensor.matmul(
                                spair[:, h, :], kt_sb[64 * h:64 * h + 64, pr, tcs],
                                qt[64 * h:64 * h + 64, :],
                                start=True, stop=True,
                            )
                        probs = probsp.tile([128, 2, 512], BF16D, name="probs", tag="probs")
                        nc.scalar.activation(probs, spair, EXP, scale=0.125)
                        nc.vector.tensor_mul(probs, probs, nmt)
                        probs_list.append(probs)
                        if prev is not None and t % 4 == 3:
                            beta_phase(*prev, t // 4)
                    prev = (c, pr, probs_list, attnT_cur)
                for j in range(4):
                    beta_phase(*prev, j)
                for k in range(4):
                    for dch in range(2):
                        outproj_group(prev[0], prev[3], k, dch)

    nc.compile()
    return nc


_NC_CACHE = {}


def _get_nc(seq=SEQ):
    if seq not in _NC_CACHE:
        _NC_CACHE[seq] = build_nc(seq)
    return _NC_CACHE[seq]


def make_core_inputs(q, k, v, mask, W_q, b_q, W_o, seq=SEQ):
    """Build the 8 per-core input maps (host-side shard + layout)."""
    NT = seq // 128
    NC_ = seq // 512
    in_maps = []
    notm_all = (~np.asarray(mask)).astype(BF16)  # [B, 16, sq, sk]
    for core in range(8):
        b, gi = divmod(core, 2)
        cols = slice(gi * DG, (gi + 1) * DG)
        xqT = np.ascontiguousarray(np.asarray(q[b], np.float32).T).astype(BF16)
        xkT = np.ascontiguousarray(np.asarray(k[b], np.float32).T).astype(BF16)
        xvT = np.ascontiguousarray(np.asarray(v[b], np.float32).T).astype(BF16)
        wqc = np.ascontiguousarray(np.asarray(W_q, np.float32)[:, cols]).astype(BF16)
        bqc = np.ascontiguousarray(np.asarray(b_q, np.float32)[cols])
        woc = np.ascontiguousarray(np.asarray(W_o, np.float32)[cols, :]).astype(BF16)
        nmc = notm_all[b, gi * HG:(gi + 1) * HG]  # [8, sq, sk] bf16
        # [8h, sq, sk] -> [pair, c, t, p(sk), h(in pair), j(sq in chunk)]
        nmc = np.ascontiguousarray(
            nmc.reshape(4, 2, NC_, 512, NT, 128).transpose(0, 2, 4, 5, 1, 3)
        )
        in_maps.append({
            "xqT": xqT, "xkT": xkT, "xvT": xvT,
            "wq": wqc, "bq": bqc, "wo": woc, "nm": nmc,
        })
    return in_maps


def kernel(q, k, v, mask, W_q, b_q, W_o, b_o):
    nc = _get_nc(SEQ)
    in_maps = make_core_inputs(q, k, v, mask, W_q, b_q, W_o, SEQ)
    res = run_bass_kernel_spmd(nc, in_maps, core_ids=list(range(8)))
    out = np.empty((BATCH, SEQ, D_MODEL), np.float32)
    bo = np.asarray(b_o, np.float32)
    for b in range(BATCH):
        out[b] = (res.results[2 * b]["out"].astype(np.float32)
                  + res.results[2 * b + 1]["out"].astype(np.float32) + bo)
    return out


# revision 3
# speedup vs baseline: 1.6236x; 1.0098x over previous
"""MultiHeadAttention Trainium2 kernel, v2.

Sharding: 8 cores = 4 batches x 2 head-groups (8 heads each).
Per core (batch b, head group gi):
  Q = q[b] @ Wq[:, gi*512:+512] + bq_g   (same fc applied to k, v)
  per head: softmax(QK^T/8 with mask) @ V
  partial_out = attn @ Wo[gi*512:+512, :]
Host sums the two partial outputs per batch and adds b_o.

v2 design (vs v1 baseline):
  - all inputs bf16 (halves x DMA; DMA_ENGINES is a serialized shared
    resource in the cost model, ~294us in v1)
  - x loaded in column-chunk tiles [128, 8db, 512] (24KB SBUF, one pool)
  - scores S^T[sk, sq] as before, but attn@V runs TRANSPOSED: probs is
    the stationary operand, out acc[sq_part, 65] -> pays 65 output rows
    per matmul instead of 512 (cost model charges out free-size only).
    All 16 probs tiles of a (c,pr) stay in SBUF; the 8 (h,j) psum
    accumulation groups then run sequentially over t (one open group
    per psum bank), software-pipelined one (c,pr) behind scores/exp.
  - 65th column of V = ones accumulates the softmax denominator, which
    lands as a PER-PARTITION scalar -> normalize via
    nc.vector.reciprocal (IEEE 1/x on trn2) + tensor_scalar mult on
    DVE. Kills v1's exp(-ln(sum)) on ACT, its LoadActFuncSet thrash
    (33 table loads = 42us), and the gpsimd broadcast.
  - attnT assembled via DMA-transpose XBAR (14ns/tile) instead of PE
  - ACT engine does ONLY the exp (the hard floor: 256 x ~1038ns)
  - Q/K bias-adds moved to DVE tensor_scalar (PSUM->SBUF copy + bias)
  - out stores bf16 via gpsimd (Pool) SWDGE queue; host sums in f32
  - phase A emits K^T, Q chunk 0, V tiles 0..3 up front; remaining V
    tiles and Q chunks are emitted just-in-time inside phase B
"""

import sys

import numpy as np
import ml_dtypes

try:
    import concourse.bass as bass  # noqa: F401
except ImportError:  # pragma: no cover
    for _p in ("/opt/trn_rl_repo", "/root/.axon_site/_ro/trn_rl_repo"):
        if _p not in sys.path:
            sys.path.insert(0, _p)
    import concourse.bass as bass  # noqa: F401

import concourse.tile as tile
from concourse import bacc, mybir
from concourse.bass_utils import run_bass_kernel_spmd

BF16 = ml_dtypes.bfloat16

D_MODEL = 1024
N_HEADS = 16
BATCH = 4
SEQ = 2048
DH = 64           # head dim
HG = 8            # heads per core
DG = HG * DH      # 512, projected dim per core

F32 = mybir.dt.float32
BF16D = mybir.dt.bfloat16


def build_nc(seq=SEQ):
    """Build the per-core SPMD Bass program."""
    assert seq % 512 == 0
    NT = seq // 128       # sk tiles
    NC_ = seq // 512      # sq chunks
    NST = seq // 128      # s tiles for V

    nc = bacc.Bacc(None, target_bir_lowering=False)

    xqT = nc.dram_tensor("xqT", [D_MODEL, seq], BF16D, kind="ExternalInput")
    xkT = nc.dram_tensor("xkT", [D_MODEL, seq], BF16D, kind="ExternalInput")
    xvT = nc.dram_tensor("xvT", [D_MODEL, seq], BF16D, kind="ExternalInput")
    wq = nc.dram_tensor("wq", [D_MODEL, DG], BF16D, kind="ExternalInput")
    bq = nc.dram_tensor("bq", [DG], F32, kind="ExternalInput")
    wo = nc.dram_tensor("wo", [DG, D_MODEL], BF16D, kind="ExternalInput")
    # notmask, transposed + tiled: [pair, c, t, p(sk in tile), h(in pair), j(sq in chunk)]
    nm = nc.dram_tensor("nm", [4, NC_, NT, 128, 2, 512], BF16D, kind="ExternalInput")
    idm = nc.dram_tensor("idm", [128, 128], BF16D, kind="ExternalInput")
    out = nc.dram_tensor("out", [seq, D_MODEL], BF16D, kind="ExternalOutput")

    EXP = mybir.ActivationFunctionType.Exp
    ADD = mybir.AluOpType.add
    MULT = mybir.AluOpType.mult

    with tile.TileContext(nc) as tc:
        with tc.tile_pool(name="persist", bufs=1) as persist:
            kt_sb = persist.tile([128, 4, seq], BF16D, name="kt_sb")
            v_sb = persist.tile([128, NST, HG, DH + 1], BF16D, name="v_sb")
            wo_sb = persist.tile([128, 4, D_MODEL], BF16D, name="wo_sb")
            wq_sb = persist.tile([128, 8, DG], BF16D, name="wq_sb")
            bq_sb = persist.tile([128, 4], F32, name="bq_sb")
            bqrep = persist.tile([128, HG, DH], F32, name="bqrep")

            with tc.tile_pool(name="xkp", bufs=4) as xkp, \
                 tc.tile_pool(name="xqp", bufs=2) as xqp, \
                 tc.tile_pool(name="xvp", bufs=2) as xvp, \
                 tc.tile_pool(name="qp", bufs=3) as qp, \
                 tc.tile_pool(name="ps512", bufs=2, space="PSUM") as ps512, \
                 tc.tile_pool(name="nmp", bufs=6) as nmp, \
                 tc.tile_pool(name="probsp", bufs=33) as probsp, \
                 tc.tile_pool(name="recp", bufs=3) as recp, \
                 tc.tile_pool(name="asbp", bufs=3) as asbp, \
                 tc.tile_pool(name="attnTp", bufs=2) as attnTp, \
                 tc.tile_pool(name="osbp", bufs=3) as osbp, \
                 tc.tile_pool(name="spairp", bufs=2, space="PSUM") as spairp, \
                 tc.tile_pool(name="accp", bufs=2, space="PSUM") as accp:

                # ---- weights / bias loads (scalar hwdge + gpsimd queues; keep
                # the sync queue free so xk0/xq0 hit the DMA engines early) ----
                nc.scalar.dma_start(out=wq_sb, in_=wq.rearrange("(n p) m -> p n m", p=128))
                nc.scalar.dma_start(out=bq_sb, in_=bq.rearrange("(t p) -> p t", p=128))
                _bqap = bq[:].rearrange("(g e) -> g e", g=HG)
                nc.gpsimd.dma_start(out=bqrep, in_=bass.AP(
                    tensor=_bqap.tensor, offset=_bqap.offset,
                    ap=[[0, 128]] + [list(d) for d in _bqap.ap]))
                nc.gpsimd.dma_start(out=wo_sb, in_=wo.rearrange("(n p) m -> p n m", p=128))
                nc.vector.memset(v_sb[:, :, :, DH:DH + 1], 1.0)

                def load_x_chunk(pool, xh, c, pref):
                    """[128 p(D in block), 8 db, 512 s] from xT[:, c*512:+512]."""
                    xt = pool.tile([128, 8, 512], BF16D, name=f"{pref}{c}", tag="x")
                    nc.sync.dma_start(
                        out=xt,
                        in_=xh[:, c * 512:(c + 1) * 512].rearrange(
                            "(n p) m -> p n m", p=128),
                    )
                    return xt

                def emit_k(xt, dt, c):
                    """Project K^T dt-slice of one 512-col chunk into kt_sb."""
                    ps = ps512.tile([128, 512], F32, name="ps", tag="ps")
                    for db in range(8):
                        nc.tensor.matmul(
                            ps, wq_sb[:, db, dt * 128:(dt + 1) * 128],
                            xt[:, db, :], start=(db == 0), stop=(db == 7),
                        )
                    nc.vector.tensor_scalar(
                        kt_sb[:, dt, c * 512:(c + 1) * 512], ps,
                        bq_sb[:, dt:dt + 1], None, ADD,
                    )

                def emit_q(xt, dt, c):
                    """Project Q^T dt-slice for chunk c into a fresh q tile."""
                    ps = ps512.tile([128, 512], F32, name="ps", tag="ps")
                    for db in range(8):
                        nc.tensor.matmul(
                            ps, wq_sb[:, db, dt * 128:(dt + 1) * 128],
                            xt[:, db, :], start=(db == 0), stop=(db == 7),
                        )
                    qt = qp.tile([128, 512], BF16D, name="qt", tag="qt")
                    nc.vector.tensor_scalar(qt, ps, bq_sb[:, dt:dt + 1], None, ADD)
                    return qt

                def emit_v(xt, st):
                    """Project V s-tile st: v_sb[:, st, :, 0:64]."""
                    i = st % 4
                    ps = ps512.tile([128, 512], F32, name="ps", tag="ps")
                    for db in range(8):
                        nc.tensor.matmul(
                            ps, xt[:, db, i * 128:(i + 1) * 128], wq_sb[:, db, :],
                            start=(db == 0), stop=(db == 7),
                        )
                    nc.vector.tensor_add(
                        v_sb[:, st, :, 0:DH],
                        ps[:, :].rearrange("p (g e) -> p g e", g=HG),
                        bqrep,
                    )

                ident = persist.tile([128, 128], BF16D, name="ident")
                nc.gpsimd.dma_start(out=ident, in_=idm[:, :])

                def beta_phase(c, pr, probs_list, attnT, j, pe_transpose=False):
                    """attn@V j-phase: 2 psum groups (h0, h1) over all t,
                    then normalize + transpose into attnT.  Steady state uses
                    the DMA-transpose XBAR (latency hidden); the final pair
                    uses a PE transpose + DVE copy to shorten the drain."""
                    asb = asbp.tile([128, 2, DH], BF16D, name="asb", tag="asb")
                    for h in range(2):
                        acc = accp.tile([128, 512], F32, name="acc", tag="acc")
                        for t in range(NT):
                            nc.tensor.matmul(
                                acc[:, 0:DH + 1],
                                probs_list[t][:, h, j * 128:(j + 1) * 128],
                                v_sb[:, t, 2 * pr + h, :],
                                start=(t == 0), stop=(t == NT - 1),
                            )
                        recip = recp.tile([128, 1], F32, name="recip", tag="recip")
                        nc.vector.reciprocal(recip, acc[:, DH:DH + 1])
                        nc.vector.tensor_scalar(
                            asb[:, h, :], acc[:, 0:DH], recip[:, :], None, MULT,
                        )
                    if pe_transpose:
                        tps = ps512.tile([128, 512], BF16D, name="tps", tag="ps")
                        nc.tensor.transpose(tps[:, 0:128], asb[:, :, :], ident)
                        nc.vector.tensor_copy(
                            attnT[:, pr, j * 128:(j + 1) * 128], tps[:, 0:128])
                    else:
                        nc.sync.dma_start(
                            out=attnT[:, pr, j * 128:(j + 1) * 128],
                            in_=asb, transpose=True,
                        )

                def outproj_group(c, attnT, k, dch):
                    row = c * 512 + k * 128
                    ops = ps512.tile([128, 512], F32, name="ops", tag="ps")
                    for pr in range(4):
                        nc.tensor.matmul(
                            ops, attnT[:, pr, k * 128:(k + 1) * 128],
                            wo_sb[:, pr, dch * 512:(dch + 1) * 512],
                            start=(pr == 0), stop=(pr == 3),
                        )
                    osb = osbp.tile([128, 512], BF16D, name="osb", tag="osb")
                    nc.vector.tensor_copy(osb, ops)
                    nc.gpsimd.dma_start(
                        out=out[row:row + 128, dch * 512:(dch + 1) * 512],
                        in_=osb,
                    )

                # ---- prefix: minimal work before the first score ----
                xk_tiles = {0: load_x_chunk(xkp, xkT, 0, "xk")}
                xq_tiles = {0: load_x_chunk(xqp, xqT, 0, "xq")}
                xv_tiles = {0: load_x_chunk(xvp, xvT, 0, "xv")}
                emit_k(xk_tiles[0], 0, 0)
                q_next = emit_q(xq_tiles[0], 0, 0)

                # ---- phase B: everything else JIT-interleaved per slot ----
                # pair index i = c*4 + pr, c-major.  Per-slot JIT calendar:
                #   c0: K[dt=pr, ch0] at pair start, ch 1-3 at slots 3,7,11;
                #       pr0 also carries V st0-15 (one per slot).
                #   all pairs: Q for the next pair at slot 13 (xq staged at 9).
                #   outproj(c-1) groups at slots 2,6,10,14 of pr1 and pr2.
                #   beta j-phases of the previous pair at slots 3,7,11,15.
                prev = None  # (c, pr, probs_list, attnT)
                attnT_cur = None
                attnT_hist = {}
                for i in range(NC_ * 4):
                    c, pr = divmod(i, 4)
                    if pr == 0:
                        attnT_cur = attnTp.tile([128, 4, 512], BF16D,
                                                name="attnT", tag="attnT")
                        attnT_hist[c] = attnT_cur
                    qt = q_next
                    probs_list = []
                    for t in range(NT):
                        # --- main per-slot pipeline ---
                        nmt = nmp.tile([128, 2, 512], BF16D, name="nmt", tag="nmt")
                        nc.sync.dma_start(out=nmt, in_=nm[pr, c, t])
                        spair = spairp.tile([128, 2, 512], F32, name="spair", tag="spair")
                        tcs = slice(t * 128, (t + 1) * 128)
                        for h in range(2):
                            nc.tensor.matmul(
                                spair[:, h, :], kt_sb[64 * h:64 * h + 64, pr, tcs],
                                qt[64 * h:64 * h + 64, :],
                                start=True, stop=True,
                            )
                        probs = probsp.tile([128, 2, 512], BF16D, name="probs", tag="probs")
                        nc.scalar.activation(probs, spair, EXP, scale=0.125)
                        nc.vector.tensor_mul(probs, probs, nmt)
                        probs_list.append(probs)
                        if prev is not None and t % 4 == 1:
                            beta_phase(*prev, t // 4)
                        # --- JIT calendar for this slot (after the score so
                        # the lead-in and spair waits aren't delayed) ---
                        if c == 0 and pr == 0:
                            if t in (0, 4, 8):
                                ch = t // 4 + 1
                                xk_tiles[ch] = load_x_chunk(xkp, xkT, ch, "xk")
                                if ch < 3:
                                    xv_tiles[ch] = load_x_chunk(xvp, xvT, ch, "xv")
                            if t == 9:
                                xv_tiles[3] = load_x_chunk(xvp, xvT, 3, "xv")
                            emit_v(xv_tiles[t // 4], t)
                        if c == 0 and t in (3, 7, 11):
                            emit_k(xk_tiles[t // 4 + 1], pr, t // 4 + 1)
                        if c == 0 and pr < 3 and t == 12:
                            # prefetch next pr's K chunk-0 slice out of the
                            # pair-boundary critical path
                            emit_k(xk_tiles[0], pr + 1, 0)
                        if t == 9 and i + 1 < NC_ * 4 and (i + 1) % 4 == 0:
                            nc2 = (i + 1) // 4
                            xq_tiles[nc2] = load_x_chunk(xqp, xqT, nc2, "xq")
                        if t == 14 and i + 1 < NC_ * 4:
                            c2, pr2 = divmod(i + 1, 4)
                            q_next = emit_q(xq_tiles[c2], pr2, c2)
                        if c >= 1 and pr >= 1 and t in (3, 7, 11) and (pr, t) != (3, 11):
                            g = (pr - 1) * 3 + t // 4  # 0..7
                            outproj_group(c - 1, attnT_hist[c - 1], g // 2, g % 2)
                    prev = (c, pr, probs_list, attnT_cur)
                # tail: PE-transpose path + stagger to shorten the drain
                beta_phase(*prev, 0, pe_transpose=True)
                for j in range(1, 4):
                    beta_phase(*prev, j, pe_transpose=True)
                    outproj_group(prev[0], prev[3], j - 1, 0)
                    outproj_group(prev[0], prev[3], j - 1, 1)
                outproj_group(prev[0], prev[3], 3, 0)
                outproj_group(prev[0], prev[3], 3, 1)

    nc.compile()
    return nc


_NC_CACHE = {}


def _get_nc(seq=SEQ):
    if seq not in _NC_CACHE:
        _NC_CACHE[seq] = build_nc(seq)
    return _NC_CACHE[seq]


def make_core_inputs(q, k, v, mask, W_q, b_q, W_o, seq=SEQ):
    """Build the 8 per-core input maps (host-side shard + layout)."""
    NT = seq // 128
    NC_ = seq // 512
    in_maps = []
    notm_all = (~np.asarray(mask)).astype(BF16)  # [B, 16, sq, sk]
    for core in range(8):
        b, gi = divmod(core, 2)
        cols = slice(gi * DG, (gi + 1) * DG)
        xqT = np.ascontiguousarray(np.asarray(q[b], np.float32).T).astype(BF16)
        xkT = np.ascontiguousarray(np.asarray(k[b], np.float32).T).astype(BF16)
        xvT = np.ascontiguousarray(np.asarray(v[b], np.float32).T).astype(BF16)
        wqc = np.ascontiguousarray(np.asarray(W_q, np.float32)[:, cols]).astype(BF16)
        bqc = np.ascontiguousarray(np.asarray(b_q, np.float32)[cols])
        woc = np.ascontiguousarray(np.asarray(W_o, np.float32)[cols, :]).astype(BF16)
        nmc = notm_all[b, gi * HG:(gi + 1) * HG]  # [8, sq, sk] bf16
        # [8h, sq, sk] -> [pair, c, t, p(sk), h(in pair), j(sq in chunk)]
        nmc = np.ascontiguousarray(
            nmc.reshape(4, 2, NC_, 512, NT, 128).transpose(0, 2, 4, 5, 1, 3)
        )
        in_maps.append({
            "xqT": xqT, "xkT": xkT, "xvT": xvT,
            "wq": wqc, "bq": bqc, "wo": woc, "nm": nmc,
            "idm": np.eye(128, dtype=BF16),
        })
    return in_maps


def kernel(q, k, v, mask, W_q, b_q, W_o, b_o):
    nc = _get_nc(SEQ)
    in_maps = make_core_inputs(q, k, v, mask, W_q, b_q, W_o, SEQ)
    res = run_bass_kernel_spmd(nc, in_maps, core_ids=list(range(8)))
    out = np.empty((BATCH, SEQ, D_MODEL), np.float32)
    bo = np.asarray(b_o, np.float32)
    for b in range(BATCH):
        out[b] = (res.results[2 * b]["out"].astype(np.float32)
                  + res.results[2 * b + 1]["out"].astype(np.float32) + bo)
    return out
